# revision 26
# baseline (speedup 1.0000x reference)
"""DensityExtractor (NeRF volume-rendering weights) Bass kernel for 8 TRN2 cores.

reference:
  dists[s] = d[s+1]-d[s] (last 1e10), scaled by |ray_dir|
  alpha = 1 - exp(-relu(rf[...,3]) * dists)
  weights = alpha * cumprod_exclusive(1 - alpha + 1e-10)

Strategy: fully data-parallel over rays (65536 -> 8 x 8192), layout
[ray partition, sample free].  DMA blocks of 1024 rays packed [128, 1024]
with ray = 1024*b + 8*r + k; compute runs on 512-col halves.

Per half [128, 512] (4 ray groups of S=128):
  dd   = shifted diff of packed depth; group-last cols memset to 1e10  (Pool)
  m    = (sigma max 0) * dd   -- scalar_tensor_tensor, sigma strided   (DVE)
  E2   = [128,513] staging: E2[c+1] = exp(-dn*m[c]) (4 ACT calls, per-
         partition scale = -|dir|); cols {0,128,256,384} memset to 0
  T2   = ONE tensor_tensor_scan over [128,513]:
             state = E2[c]*state + b[c],  b = 1.0 at cols {0,128,256,384}
                                              (group reset), 1e-10 else
         => T2[c] = exclusive transmittance prod_{j<c}(e_j + 1e-10) with
         per-group restart baked into the scan operands -- no per-group
         scan calls, no boundary fixup pass.
  w[c] = T2[c] - T2[c+1]  (= alpha*T_excl - eps*T_excl)                (TT sub)

scan and the w-sub alternate between DVE and Pool per half to balance.
rf loads alternate sync/scalar HW queues; dv loads + stores on sync
(stores deferred by 2 halves so load dispatches stay ahead).
"""

import sys

for _p in ("/opt/trn_rl_repo", "/root/.axon_site/_ro/trn_rl_repo"):
    if _p not in sys.path:
        sys.path.append(_p)

from contextlib import ExitStack

import numpy as np

import concourse.bass as bass
import concourse.tile as tile
from concourse import bacc, mybir
from concourse.bass_utils import run_bass_kernel_spmd

FP = mybir.dt.float32
OP = mybir.AluOpType
AF = mybir.ActivationFunctionType
N_CORES = 8
N_RAYS = 65536
S = 128
ONE_E_10 = 1.0e10
EPS = 1.0e-10


def build_module(n_rays=N_RAYS // N_CORES, blk=1024, bufs=6, cbufs=8):
    nblk = n_rays // blk
    K = blk // 128  # ray groups per DMA block
    W = K * S  # free width of packed tiles
    H = W // 512  # compute halves per block
    assert n_rays % blk == 0 and W % 512 == 0

    nc = bacc.Bacc("TRN2", target_bir_lowering=False, debug=False)
    rf = nc.dram_tensor("radiance_field", [n_rays, S, 4], FP, kind="ExternalInput").ap()
    dv = nc.dram_tensor("depth_values", [n_rays, S], FP, kind="ExternalInput").ap()
    rd = nc.dram_tensor("ray_directions", [n_rays, 3], FP, kind="ExternalInput").ap()
    out = nc.dram_tensor("weights", [n_rays, S], FP, kind="ExternalOutput").ap()

    with tile.TileContext(nc) as tc, ExitStack() as ctx:
        consts = ctx.enter_context(tc.tile_pool(name="consts", bufs=1))
        rfp = ctx.enter_context(tc.tile_pool(name="rf", bufs=bufs))
        dvp = ctx.enter_context(tc.tile_pool(name="dv", bufs=cbufs))
        ddp = ctx.enter_context(tc.tile_pool(name="dd", bufs=cbufs))
        mpp = ctx.enter_context(tc.tile_pool(name="m", bufs=cbufs))
        epp = ctx.enter_context(tc.tile_pool(name="e", bufs=cbufs))
        tpp = ctx.enter_context(tc.tile_pool(name="T", bufs=cbufs))
        wpp = ctx.enter_context(tc.tile_pool(name="w", bufs=cbufs))

        # work units (b, g0, G): G consecutive ray groups of block b.
        # First and last blocks use narrow units so the pipeline fills and
        # drains in half the time; the DMA-saturated middle uses wide ones.
        units = []
        for b in range(nblk):
            widths = [4, 2, 2] if b == nblk - 1 else [4] * (K // 4)
            g0 = 0
            for g in widths:
                units.append((b, g0, g))
                g0 += g
        n_half = len(units)

        def rf_ap(b, g0, G):
            r0 = b * blk
            return rf[r0 : r0 + blk].rearrange("(r k) s c -> r k (s c)", k=K)[
                :, g0 : g0 + G, :
            ]

        def dv_ap(b, g0, G):
            r0 = b * blk
            return dv[r0 : r0 + blk, :].rearrange("(r k) s -> r k s", k=K)[
                :, g0 : g0 + G, :
            ]

        def out_ap(b, g0, G):
            r0 = b * blk
            return out[r0 : r0 + blk, :].rearrange("(r k) s -> r k s", k=K)[
                :, g0 : g0 + G, :
            ]

        # ray_directions load goes FIRST, on the otherwise-idle DVE queue, so
        # the dir-norm prologue finishes under the first rf loads instead of
        # queueing behind a 1MB rf transfer on a shared HW queue ring.
        # keep (k c) fused so each descriptor covers a 96B contiguous run
        # (splitting on c gives 8192 x 12B descriptors and a ~8us DMA).
        # First in the sync queue's FIFO: the dir-norm chain gates the first
        # exp, and the SWDGE path starts ~7us later than the HW queues.
        rdt = consts.tile([128, nblk * K * 3], FP, tag="rdt")
        nc.sync.dma_start(
            rdt[:].rearrange("r (b kc) -> r b kc", b=nblk),
            rd.rearrange("(b r k) c -> r b (k c)", r=128, k=K),
        )

        # issue the first loads before the prologue compute so the DMA
        # engines are busy from t=0
        PREFETCH = 4
        rf_tiles = {}
        dv_tiles = {}

        def issue_loads(i):
            b, g0, G = units[i]
            rf_p = rfp.tile([128, 512 * G], FP, tag="rf")
            (nc.sync if i % 2 == 0 else nc.scalar).dma_start(
                rf_p[:].rearrange("r (k f) -> r k f", k=G), rf_ap(b, g0, G)
            )
            dv_p = dvp.tile([128, S * G], FP, tag="dv")
            nc.sync.dma_start(
                dv_p[:].rearrange("r (k s) -> r k s", k=G), dv_ap(b, g0, G)
            )
            rf_tiles[i] = rf_p
            dv_tiles[i] = dv_p

        for i in range(min(PREFETCH, n_half)):
            issue_loads(i)

        # --- dir_norm prologue ---
        # dnneg[r, K*b+k] = -|ray_dir| of ray blk*b + K*r + k (block packing)
        sq = consts.tile([128, nblk * K * 3], FP, tag="sq")
        nc.vector.tensor_mul(sq[:], rdt[:], rdt[:])
        sq3 = sq[:].rearrange("r (t c) -> r t c", c=3)
        dn2 = consts.tile([128, nblk * K], FP, tag="dn2")
        nc.vector.tensor_add(dn2[:], sq3[:, :, 0], sq3[:, :, 1])
        nc.vector.tensor_add(dn2[:], dn2[:], sq3[:, :, 2])
        dnneg = consts.tile([128, nblk * K], FP, tag="dnneg")
        nc.scalar.activation(dnneg[:], dn2[:], AF.Sqrt)
        nc.vector.tensor_scalar_mul(dnneg[:], dnneg[:], -1.0)

        # scan add-operand: 1.0 at the 4 group-reset cols, eps elsewhere.
        # Each group gets S+1 scan columns: a reset col (E2=0, b=1) followed
        # by its S exp values, so the group's final transmittance (needed by
        # w[S-1]) is not clobbered by the next group's reset.
        S1 = S + 1
        WS = 4 * S1  # scan width per half
        bconst = consts.tile([128, WS], FP, tag="bconst")
        nc.vector.memset(bconst[:], EPS)
        nc.vector.memset(
            bconst[:].rearrange("r (k s) -> r k s", s=S1)[:, :, 0], 1.0
        )

        pending_stores = []

        def flush_store():
            o_ap, w_tile = pending_stores.pop(0)
            nc.sync.dma_start(o_ap, w_tile)

        for i in range(n_half):
            b, g0, G = units[i]
            WG = S * G
            if i + PREFETCH < n_half:
                issue_loads(i + PREFETCH)
            rf_p = rf_tiles.pop(i)
            dv_p = dv_tiles.pop(i)

            # dd: shifted diff; every group-last col must be 1e10 (ref
            # semantics for the final interval; also kills cross-group cols)
            dd = ddp.tile([128, WG], FP, tag="dd")
            nc.gpsimd.tensor_sub(dd[:, 0 : WG - 1], dv_p[:, 1:WG], dv_p[:, 0 : WG - 1])
            nc.gpsimd.memset(
                dd[:].rearrange("r (k s) -> r k s", s=S)[:, :, S - 1], ONE_E_10
            )

            # m = relu(sigma) * dd, sigma strided from packed rf
            # (scalar_tensor_tensor and the scan are DVE-only in codegen)
            sig = rf_p[:].rearrange("r (x c) -> r x c", c=4)[:, :, 3]
            m_ = mpp.tile([128, WG], FP, tag="m")
            nc.vector.scalar_tensor_tensor(m_[:], sig, 0.0, dd[:], OP.max, OP.mult)

            # E2[129k+1+c] = exp(-dn*m[128k+c]); reset cols {129k} -> 0.
            # The reset cols are zeroed on ACT too (Copy with scale=0) so the
            # scan has a single producer engine for e_ (fewer sem edges).
            e_ = epp.tile([128, S1 * G], FP, tag="e")
            ecols = e_[:].rearrange("r (k s) -> r k s", s=S1)[:, :, 0]
            bcols = bconst[:, 0 : S1 * G].rearrange("r (k s) -> r k s", s=S1)[:, :, 0]
            nc.scalar.activation(ecols, bcols, AF.Copy, scale=0.0)
            for k in range(G):
                c = K * b + g0 + k
                nc.scalar.activation(
                    e_[:, S1 * k + 1 : S1 * k + 1 + S],
                    m_[:, S * k : S * (k + 1)],
                    AF.Exp,
                    scale=dnneg[:, c : c + 1],
                )

            # exclusive transmittance in ONE scan: state = E2*state + b
            # (tensor_tensor_scan is DVE-only; Pool is rejected by codegen)
            T_ = tpp.tile([128, S1 * G], FP, tag="T")
            nc.vector.tensor_tensor_scan(
                T_[:], e_[:], bconst[:, 0 : S1 * G], 1.0, OP.mult, OP.add
            )

            # w[c] = T2[129k+c] - T2[129k+c+1]
            w_ = wpp.tile([128, WG], FP, tag="w")
            T3 = T_[:].rearrange("r (k s) -> r k s", s=S1)
            nc.gpsimd.tensor_sub(
                w_[:].rearrange("r (k s) -> r k s", s=S),
                T3[:, :, 0:S],
                T3[:, :, 1 : S + 1],
            )

            pending_stores.append(
                (out_ap(b, g0, G), w_[:].rearrange("r (k s) -> r k s", s=S))
            )
            if len(pending_stores) > 2:
                flush_store()

        while pending_stores:
            flush_store()

    nc.compile()
    return nc


_NC_CACHE = {}


def get_module(n_rays=N_RAYS // N_CORES, **kw):
    key = (n_rays, tuple(sorted(kw.items())))
    if key not in _NC_CACHE:
        _NC_CACHE[key] = build_module(n_rays, **kw)
    return _NC_CACHE[key]


def run_spmd(radiance_field, depth_values, ray_directions, trace=False, **kw):
    nc = get_module(**kw)
    per = radiance_field.shape[0] // N_CORES
    in_maps = []
    for i in range(N_CORES):
        s = slice(i * per, (i + 1) * per)
        in_maps.append(
            {
                "radiance_field": np.ascontiguousarray(radiance_field[s]),
                "depth_values": np.ascontiguousarray(depth_values[s]),
                "ray_directions": np.ascontiguousarray(ray_directions[s]),
            }
        )
    res = run_bass_kernel_spmd(nc, in_maps, list(range(N_CORES)), trace=trace)
    out = np.concatenate([r["weights"] for r in res.results], axis=0)
    return out, res


def kernel(radiance_field, depth_values, ray_directions):
    out, _ = run_spmd(
        np.asarray(radiance_field, dtype=np.float32),
        np.asarray(depth_values, dtype=np.float32),
        np.asarray(ray_directions, dtype=np.float32),
    )
    return out


# revision 27
# speedup vs baseline: 1.1113x; 1.1113x over previous
"""DensityExtractor (NeRF volume-rendering weights) Bass kernel for 8 TRN2 cores.

reference:
  dists[s] = d[s+1]-d[s] (last 1e10), scaled by |ray_dir|
  alpha = 1 - exp(-relu(rf[...,3]) * dists)
  weights = alpha * cumprod_exclusive(1 - alpha + 1e-10)

Strategy: fully data-parallel over rays (65536 -> 8 x 8192), layout
[ray partition, sample free].  DMA blocks of 1024 rays packed [128, 1024]
with ray = 1024*b + 8*r + k; compute runs on 512-col halves.

Per half [128, 512] (4 ray groups of S=128):
  dd   = shifted diff of packed depth; group-last cols memset to 1e10  (Pool)
  m    = (sigma max 0) * dd   -- scalar_tensor_tensor, sigma strided   (DVE)
  E2   = [128,513] staging: E2[c+1] = exp(-dn*m[c]) (4 ACT calls, per-
         partition scale = -|dir|); cols {0,128,256,384} memset to 0
  T2   = ONE tensor_tensor_scan over [128,513]:
             state = E2[c]*state + b[c],  b = 1.0 at cols {0,128,256,384}
                                              (group reset), 1e-10 else
         => T2[c] = exclusive transmittance prod_{j<c}(e_j + 1e-10) with
         per-group restart baked into the scan operands -- no per-group
         scan calls, no boundary fixup pass.
  w[c] = T2[c] - T2[c+1]  (= alpha*T_excl - eps*T_excl)                (TT sub)

scan and the w-sub alternate between DVE and Pool per half to balance.
rf loads alternate sync/scalar HW queues; dv loads + stores on sync
(stores deferred by 2 halves so load dispatches stay ahead).
"""

import sys

for _p in ("/opt/trn_rl_repo", "/root/.axon_site/_ro/trn_rl_repo"):
    if _p not in sys.path:
        sys.path.append(_p)

from contextlib import ExitStack

import numpy as np

import concourse.bass as bass
import concourse.tile as tile
from concourse import bacc, mybir
from concourse.bass_utils import run_bass_kernel_spmd

FP = mybir.dt.float32
OP = mybir.AluOpType
AF = mybir.ActivationFunctionType
N_CORES = 8
N_RAYS = 65536
S = 128
ONE_E_10 = 1.0e10
EPS = 1.0e-10


def build_module(n_rays=N_RAYS // N_CORES, blk=1024, bufs=6, cbufs=8):
    nblk = n_rays // blk
    K = blk // 128  # ray groups per DMA block
    W = K * S  # free width of packed tiles
    H = W // 512  # compute halves per block
    assert n_rays % blk == 0 and W % 512 == 0

    nc = bacc.Bacc("TRN2", target_bir_lowering=False, debug=False)
    rf = nc.dram_tensor("radiance_field", [n_rays, S, 4], FP, kind="ExternalInput").ap()
    dv = nc.dram_tensor("depth_values", [n_rays, S], FP, kind="ExternalInput").ap()
    rd = nc.dram_tensor("ray_directions", [n_rays, 3], FP, kind="ExternalInput").ap()
    out = nc.dram_tensor("weights", [n_rays, S], FP, kind="ExternalOutput").ap()

    with tile.TileContext(nc) as tc, ExitStack() as ctx:
        consts = ctx.enter_context(tc.tile_pool(name="consts", bufs=1))
        rfp = ctx.enter_context(tc.tile_pool(name="rf", bufs=bufs))
        dvp = ctx.enter_context(tc.tile_pool(name="dv", bufs=cbufs))
        ddp = ctx.enter_context(tc.tile_pool(name="dd", bufs=cbufs))
        mpp = ctx.enter_context(tc.tile_pool(name="m", bufs=cbufs))
        epp = ctx.enter_context(tc.tile_pool(name="e", bufs=cbufs))
        tpp = ctx.enter_context(tc.tile_pool(name="T", bufs=cbufs))
        wpp = ctx.enter_context(tc.tile_pool(name="w", bufs=cbufs))

        # work units (b, g0, G): G consecutive ray groups of block b.
        # First and last blocks use narrow units so the pipeline fills and
        # drains in half the time; the DMA-saturated middle uses wide ones.
        units = []
        for b in range(nblk):
            widths = [4] * (K // 4)
            g0 = 0
            for g in widths:
                units.append((b, g0, g))
                g0 += g
        n_half = len(units)

        def rf_ap(b, g0, G):
            r0 = b * blk
            return rf[r0 : r0 + blk].rearrange("(r k) s c -> r k (s c)", k=K)[
                :, g0 : g0 + G, :
            ]

        def dv_ap(b, g0, G):
            r0 = b * blk
            return dv[r0 : r0 + blk, :].rearrange("(r k) s -> r k s", k=K)[
                :, g0 : g0 + G, :
            ]

        def out_ap(b, g0, G):
            r0 = b * blk
            return out[r0 : r0 + blk, :].rearrange("(r k) s -> r k s", k=K)[
                :, g0 : g0 + G, :
            ]

        # ray_directions load goes FIRST, on the otherwise-idle DVE queue, so
        # the dir-norm prologue finishes under the first rf loads instead of
        # queueing behind a 1MB rf transfer on a shared HW queue ring.
        # keep (k c) fused so each descriptor covers a 96B contiguous run
        # (splitting on c gives 8192 x 12B descriptors and a ~8us DMA).
        # First in the sync queue's FIFO: the dir-norm chain gates the first
        # exp, and the SWDGE path starts ~7us later than the HW queues.
        rdt = consts.tile([128, nblk * K * 3], FP, tag="rdt")
        nc.sync.dma_start(
            rdt[:].rearrange("r (b kc) -> r b kc", b=nblk),
            rd.rearrange("(b r k) c -> r b (k c)", r=128, k=K),
        )

        # issue the first loads before the prologue compute so the DMA
        # engines are busy from t=0
        PREFETCH = 4
        rf_tiles = {}
        dv_tiles = {}

        def issue_loads(i):
            b, g0, G = units[i]
            rf_p = rfp.tile([128, 512 * G], FP, tag="rf")
            (nc.sync if i % 2 == 0 else nc.scalar).dma_start(
                rf_p[:].rearrange("r (k f) -> r k f", k=G), rf_ap(b, g0, G)
            )
            dv_p = dvp.tile([128, S * G], FP, tag="dv")
            nc.sync.dma_start(
                dv_p[:].rearrange("r (k s) -> r k s", k=G), dv_ap(b, g0, G)
            )
            rf_tiles[i] = rf_p
            dv_tiles[i] = dv_p

        for i in range(min(PREFETCH, n_half)):
            issue_loads(i)

        # --- dir_norm prologue ---
        # dnneg[r, K*b+k] = -|ray_dir| of ray blk*b + K*r + k (block packing)
        sq = consts.tile([128, nblk * K * 3], FP, tag="sq")
        nc.vector.tensor_mul(sq[:], rdt[:], rdt[:])
        sq3 = sq[:].rearrange("r (t c) -> r t c", c=3)
        dn2 = consts.tile([128, nblk * K], FP, tag="dn2")
        nc.vector.tensor_add(dn2[:], sq3[:, :, 0], sq3[:, :, 1])
        nc.vector.tensor_add(dn2[:], dn2[:], sq3[:, :, 2])
        dnneg = consts.tile([128, nblk * K], FP, tag="dnneg")
        nc.scalar.activation(dnneg[:], dn2[:], AF.Sqrt)
        nc.vector.tensor_scalar_mul(dnneg[:], dnneg[:], -1.0)

        # scan add-operand: 1.0 at the 4 group-reset cols, eps elsewhere.
        # Each group gets S+1 scan columns: a reset col (E2=0, b=1) followed
        # by its S exp values, so the group's final transmittance (needed by
        # w[S-1]) is not clobbered by the next group's reset.
        S1 = S + 1
        WS = 4 * S1  # scan width per half
        bconst = consts.tile([128, WS], FP, tag="bconst")
        nc.vector.memset(bconst[:], EPS)
        nc.vector.memset(
            bconst[:].rearrange("r (k s) -> r k s", s=S1)[:, :, 0], 1.0
        )

        pending_stores = []

        def flush_store():
            o_ap, w_tile = pending_stores.pop(0)
            nc.sync.dma_start(o_ap, w_tile)

        for i in range(n_half):
            b, g0, G = units[i]
            WG = S * G
            if i + PREFETCH < n_half:
                issue_loads(i + PREFETCH)
            rf_p = rf_tiles.pop(i)
            dv_p = dv_tiles.pop(i)

            # dd: shifted diff; every group-last col must be 1e10 (ref
            # semantics for the final interval; also kills cross-group cols)
            dd = ddp.tile([128, WG], FP, tag="dd")
            nc.gpsimd.tensor_sub(dd[:, 0 : WG - 1], dv_p[:, 1:WG], dv_p[:, 0 : WG - 1])
            nc.gpsimd.memset(
                dd[:].rearrange("r (k s) -> r k s", s=S)[:, :, S - 1], ONE_E_10
            )

            # m = relu(sigma) * dd, sigma strided from packed rf
            # (scalar_tensor_tensor and the scan are DVE-only in codegen)
            sig = rf_p[:].rearrange("r (x c) -> r x c", c=4)[:, :, 3]
            m_ = mpp.tile([128, WG], FP, tag="m")
            nc.vector.scalar_tensor_tensor(m_[:], sig, 0.0, dd[:], OP.max, OP.mult)

            # E2[129k+1+c] = exp(-dn*m[128k+c]); reset cols {129k} -> 0.
            # The reset cols are zeroed on ACT too (Copy with scale=0) so the
            # scan has a single producer engine for e_ (fewer sem edges).
            e_ = epp.tile([128, S1 * G], FP, tag="e")
            ecols = e_[:].rearrange("r (k s) -> r k s", s=S1)[:, :, 0]
            bcols = bconst[:, 0 : S1 * G].rearrange("r (k s) -> r k s", s=S1)[:, :, 0]
            nc.scalar.activation(ecols, bcols, AF.Copy, scale=0.0)
            for k in range(G):
                c = K * b + g0 + k
                nc.scalar.activation(
                    e_[:, S1 * k + 1 : S1 * k + 1 + S],
                    m_[:, S * k : S * (k + 1)],
                    AF.Exp,
                    scale=dnneg[:, c : c + 1],
                )

            # exclusive transmittance in ONE scan: state = E2*state + b
            # (tensor_tensor_scan is DVE-only; Pool is rejected by codegen)
            T_ = tpp.tile([128, S1 * G], FP, tag="T")
            nc.vector.tensor_tensor_scan(
                T_[:], e_[:], bconst[:, 0 : S1 * G], 1.0, OP.mult, OP.add
            )

            # w[c] = T2[129k+c] - T2[129k+c+1]
            w_ = wpp.tile([128, WG], FP, tag="w")
            T3 = T_[:].rearrange("r (k s) -> r k s", s=S1)
            nc.gpsimd.tensor_sub(
                w_[:].rearrange("r (k s) -> r k s", s=S),
                T3[:, :, 0:S],
                T3[:, :, 1 : S + 1],
            )

            pending_stores.append(
                (out_ap(b, g0, G), w_[:].rearrange("r (k s) -> r k s", s=S))
            )
            if len(pending_stores) > 2:
                flush_store()

        while pending_stores:
            flush_store()

    nc.compile()
    return nc


_NC_CACHE = {}


def get_module(n_rays=N_RAYS // N_CORES, **kw):
    key = (n_rays, tuple(sorted(kw.items())))
    if key not in _NC_CACHE:
        _NC_CACHE[key] = build_module(n_rays, **kw)
    return _NC_CACHE[key]


def run_spmd(radiance_field, depth_values, ray_directions, trace=False, **kw):
    nc = get_module(**kw)
    per = radiance_field.shape[0] // N_CORES
    in_maps = []
    for i in range(N_CORES):
        s = slice(i * per, (i + 1) * per)
        in_maps.append(
            {
                "radiance_field": np.ascontiguousarray(radiance_field[s]),
                "depth_values": np.ascontiguousarray(depth_values[s]),
                "ray_directions": np.ascontiguousarray(ray_directions[s]),
            }
        )
    res = run_bass_kernel_spmd(nc, in_maps, list(range(N_CORES)), trace=trace)
    out = np.concatenate([r["weights"] for r in res.results], axis=0)
    return out, res


def kernel(radiance_field, depth_values, ray_directions):
    out, _ = run_spmd(
        np.asarray(radiance_field, dtype=np.float32),
        np.asarray(depth_values, dtype=np.float32),
        np.asarray(ray_directions, dtype=np.float32),
    )
    return out
